# revision 41
# baseline (speedup 1.0000x reference)
"""Distributed Bass kernel for additive (Bahdanau-style) attention on 8 TRN2
NeuronCores.

Math (reference):
    temp_enc = enc @ Wenc^T                    [1,S,H]
    temp_dec = dec @ Wdec^T                    [1,H]
    x        = tanh(temp_dec + temp_enc)
    e        = x @ va^T                        [1,S,1]
    w        = softmax(mask ? e : -inf)        [1,S,1]
    ctx      = w^T @ enc                       [1,1,H]

Strategy: shard S across 8 cores (8192 rows each). Each core:
  - streams its enc shard from HBM once (f32), casts to bf16, transposes
    s<->h on TensorE (is_transpose matmuls; the DMA-xbar path costs ~1.2us
    of HWDGE descriptor-gen per 128x128 block on this runtime, measured),
  - temp_enc^T = WencT @ encT on TensorE (bf16, f32 accum in PSUM),
  - tanh+bias fused on ScalarE (bias = temp_dec as per-partition scalar,
    since the layout is transposed), output bf16,
  - energies e = va^T x on TensorE with va stationary ([1,512] rows), spread
    to softmax-friendly [128, chunk] columns with K=1 broadcast matmuls,
  - exp WITHOUT max subtraction (|e| <= ||va||_1 ~ 18, safely in f32),
    masked, then partial context = sum_s exp(e_s) * enc_s accumulated in
    PSUM across all 64 s-tiles,
  - ONE AllGather of [partial_ctx(512) | partial_Z(1)] (2KB per rank),
    reduced across ranks on TensorE (cheaper than AllReduce's CCE path),
  - normalize: ctx = y/Z, w_s = exp(e_s)/Z.

All host<->device tensors stay contiguous; every partition-spread (dec, va,
temp_dec, mask, attention-weight store) goes through tiny TensorE transposes
or K=1 broadcast matmuls instead of 4-byte-strided DMA descriptors.

Exactness vs reference: softmax(e) is invariant to the max subtraction up to
f32 rounding; bf16 is only used for matmul operands (f32 accumulation).
"""

import numpy as np

import ml_dtypes

import concourse.bass as bass
import concourse.mybir as mybir
import concourse.tile as tile
from concourse.bass_utils import run_bass_kernel_spmd

COLLECTIVE = "AllReduce"  # or "AllReduce"

NCORES = 8
H = 512
S = 65536
SLOC = S // NCORES  # 8192
P = 128
NST = SLOC // P  # 64 s-tiles of 128
CH = 4  # s-tiles per chunk (chunk = 512 sequence positions)
NCHUNK = NST // CH  # 16
NH = H // P  # 4 h-tiles
AR_N = 520  # 512 ctx + 1 Z + 7 pad (32B-aligned total)

F32 = mybir.dt.float32
BF16 = mybir.dt.bfloat16
AF = mybir.ActivationFunctionType
ALU = mybir.AluOpType

# ---------------------------------------------------------------------------
# walrus on this container rejects >1 sync-wait per instruction; hoist the
# excess onto same-engine NOPs placed immediately before the instruction.
_DEFAULT_LIMIT = 1


def _fix_wait_overflow(nc):
    eng_map = {
        mybir.EngineType.PE: nc.tensor,
        mybir.EngineType.DVE: nc.vector,
        mybir.EngineType.Activation: nc.scalar,
        mybir.EngineType.Pool: nc.gpsimd,
        mybir.EngineType.SP: nc.sync,
    }
    for bb in nc.main_func.blocks:
        insts = bb.instructions
        i = 0
        while i < len(insts):
            ins = insts[i]
            limit = _DEFAULT_LIMIT
            si = ins.sync_info
            waits = list(si.on_wait) if si and si.on_wait else []
            if len(waits) > limit:
                keep, extra = waits[:limit], waits[limit:]
                eng = eng_map[ins.engine]
                nops = []
                for j in range(len(extra)):
                    nop_ins = eng.nop(nofuse=True).ins
                    nop_ins.sync_info = type(si)(on_wait=[extra[j]], on_update=[])
                    for b2 in nc.main_func.blocks:
                        if nop_ins in b2.instructions:
                            b2.instructions.remove(nop_ins)
                            break
                    nops.append(nop_ins)
                ins.sync_info = type(si)(on_wait=keep, on_update=si.on_update)
                insts[i:i] = nops
                i += len(nops)
            i += 1
    return nc


# ---------------------------------------------------------------------------
def build_nc():
    nc = bass.Bass()

    enc = nc.declare_dram_parameter("enc", [SLOC, H], F32, isOutput=False)
    dec = nc.declare_dram_parameter("dec", [1, H], F32, isOutput=False)
    msk = nc.declare_dram_parameter("msk", [SLOC], F32, isOutput=False)
    wenc = nc.declare_dram_parameter("wenc", [H, H], F32, isOutput=False)
    wdec = nc.declare_dram_parameter("wdec", [H, H], F32, isOutput=False)
    va = nc.declare_dram_parameter("va", [1, H], F32, isOutput=False)
    cid_f = nc.declare_dram_parameter("cid_f", [P, 2 * P], F32, isOutput=False)
    cid_bf = nc.declare_dram_parameter("cid_bf", [P, P], BF16, isOutput=False)
    attn_out = nc.declare_dram_parameter("attn_out", [SLOC], F32, isOutput=True)
    ctx_out = nc.declare_dram_parameter("ctx_out", [1, H], F32, isOutput=True)

    ar_in = nc.dram_tensor("ar_in", [1, AR_N], F32)
    if COLLECTIVE == "AllGather":
        ag_out = nc.dram_tensor("ag_out", [NCORES, AR_N], F32, addr_space="Shared")
    else:
        ag_out = nc.dram_tensor("ag_out", [1, AR_N], F32, addr_space="Shared")

    with tile.TileContext(nc) as tc:
        with (
            tc.tile_pool(name="singles", bufs=1) as singles,
            tc.tile_pool(name="wsetup", bufs=1) as wsetup,
            tc.tile_pool(name="io", bufs=12) as io,
            tc.tile_pool(name="pt", bufs=4, space="PSUM") as pt_pool,
            tc.tile_pool(name="psum1", bufs=2, space="PSUM") as psum1,
            tc.tile_pool(name="pe", bufs=1, space="PSUM") as pe_pool,
            tc.tile_pool(name="psum_ctx", bufs=1, space="PSUM") as psum_ctx,
        ):
            # ---------------- prefetch first enc chunks + small inputs ------
            enc_pre = {}

            def load_chunk(c):
                s0 = c * CH * P
                tiles = []
                for st in range(CH):
                    t = io.tile([P, H], F32, tag="enc_f32")
                    nc.sync.dma_start(t[:], enc[s0 + st * P : s0 + (st + 1) * P, :])
                    tiles.append(t)
                return tiles

            # ---------------- small constants (DMA'd, not generated) -------
            cf = singles.tile([P, 2 * P], F32)
            nc.sync.dma_start(cf[:], cid_f[:])
            id_bf = singles.tile([P, P], BF16)
            nc.sync.dma_start(id_bf[:], cid_bf[:])

            enc_pre[0] = load_chunk(0)
            # Wenc load goes out right after chunk 0 so mm1 can start early
            we_f32 = []
            for oi in range(NH):
                t = wsetup.tile([P, H], F32, tag=f"wef32_{oi}")
                nc.sync.dma_start(t[:], wenc[oi * P : (oi + 1) * P, :])
                we_f32.append(t)
            enc_pre[1] = load_chunk(1)
            dec_nat = singles.tile([1, H], F32)
            va_nat = singles.tile([1, H], F32)
            nc.sync.dma_start(dec_nat[:], dec[:])
            nc.sync.dma_start(va_nat[:], va[:])
            mask_nat = singles.tile([NST, P], F32)
            nc.sync.dma_start(mask_nat[:], msk[:].rearrange("(j p) -> j p", p=P))

            id_f = cf[:, 0:P]
            one1 = cf[0:1, P : P + 1]
            ones_col = cf[:, P : P + 1]
            ones_row = cf[0:1, P : 2 * P]
            ones8 = cf[0:NCORES, P : P + 1]

            # ---------------- stage A: cast + transpose one chunk ----------
            def stage_a(c):
                enc_f32 = enc_pre.pop(c) if c in enc_pre else load_chunk(c)
                enc_bf = []
                for st in range(CH):
                    t = io.tile([P, H], BF16, tag="enc_bf")
                    nc.vector.tensor_copy(t[:], enc_f32[st][:])
                    enc_bf.append(t)
                encT = []
                for hi in range(NH):
                    ptt = pt_pool.tile([P, CH * P], BF16, tag="pt")
                    for st in range(CH):
                        nc.tensor.transpose(
                            ptt[:, st * P : (st + 1) * P],
                            enc_bf[st][:, hi * P : (hi + 1) * P],
                            id_bf[:],
                        )
                    t = io.tile([P, CH * P], BF16, tag="encT")
                    nc.vector.tensor_copy(t[:], ptt[:])
                    encT.append(t)
                return enc_bf, encT

            a_tiles = {0: stage_a(0), 1: stage_a(1)}

            # ---------------- remaining setup ----------------
            # Wenc -> WencT bf16 via PE transposes
            wencT = []
            for hi in range(NH):
                pw = pt_pool.tile([P, H], F32, tag="pt")
                for oi in range(NH):
                    nc.tensor.transpose(
                        pw[:, oi * P : (oi + 1) * P],
                        we_f32[oi][:, hi * P : (hi + 1) * P],
                        id_f[:],
                    )
                t = wsetup.tile([P, H], BF16, tag=f"weT_{hi}")
                nc.vector.tensor_copy(t[:], pw[:])
                wencT.append(t)

            # va columns [128, NH] bf16 via K=1 broadcast matmuls
            pc = pe_pool.tile([P, NH], F32, tag="pe")
            for hi in range(NH):
                nc.tensor.matmul(
                    pc[:, hi : hi + 1],
                    va_nat[0:1, hi * P : (hi + 1) * P],
                    one1[:],
                    start=True,
                    stop=True,
                )
            va_col = singles.tile([P, NH], BF16)
            nc.vector.tensor_copy(va_col[:], pc[:])

            # Wdec natural + temp_dec on DVE
            wd_f32 = []
            for oi in range(NH):
                t = wsetup.tile([P, H], F32, tag=f"wdf32_{oi}")
                nc.sync.dma_start(t[:], wdec[oi * P : (oi + 1) * P, :])
                wd_f32.append(t)
            pdb = pe_pool.tile([P, H], F32, tag="pe")
            nc.tensor.matmul(pdb[:], ones_row[:], dec_nat[:], start=True, stop=True)
            dec_bcast = singles.tile([P, H], F32)
            nc.vector.tensor_copy(dec_bcast[:], pdb[:])
            td_col = singles.tile([P, NH], F32)
            tt_scratch = singles.tile([P, H], F32)
            for oi in range(NH):
                nc.vector.tensor_mul(tt_scratch[:], wd_f32[oi][:], dec_bcast[:])
                nc.vector.reduce_sum(
                    td_col[:, oi : oi + 1],
                    tt_scratch[:],
                    axis=mybir.AxisListType.X,
                )

            # mask: PE-transpose [64,128] -> [128,64]
            pm = pt_pool.tile([P, NST], F32, tag="pt")
            nc.tensor.transpose(pm[:], mask_nat[:], id_f[:NST, :NST])
            mask_sb = singles.tile([P, NST], F32)
            nc.vector.tensor_copy(mask_sb[:], pm[:])

            # persistent softmax state
            expm = singles.tile([P, NST], F32)  # masked exp(e)
            w_bf16 = singles.tile([P, NST], BF16)  # bf16 copy for ctx matmul
            ctx_psum = psum_ctx.tile([1, H], F32)
            ar_sb = singles.tile([1, AR_N], F32)
            nc.vector.memset(ar_sb[:], 0.0)

            # ---------------- stage B: matmuls + softmax pieces ------------
            def stage_b(c, enc_bf, encT):
                x_bf = []
                for oi in range(NH):
                    p1 = psum1.tile([P, CH * P], F32, tag="psum1")
                    for hi in range(NH):
                        nc.tensor.matmul(
                            p1[:],
                            wencT[hi][:, oi * P : (oi + 1) * P],
                            encT[hi][:],
                            start=(hi == 0),
                            stop=(hi == NH - 1),
                        )
                    xt = io.tile([P, CH * P], BF16, tag="x_bf")
                    nc.scalar.activation(
                        xt[:], p1[:], AF.Tanh, bias=td_col[:, oi : oi + 1]
                    )
                    x_bf.append(xt)
                # energies into [128, CH] columns (x blocks stationary)
                pe2 = pe_pool.tile([P, CH], F32, tag="pe")
                for st in range(CH):
                    for oi in range(NH):
                        nc.tensor.matmul(
                            pe2[:, st : st + 1],
                            x_bf[oi][:, st * P : (st + 1) * P],
                            va_col[:, oi : oi + 1],
                            start=(oi == 0),
                            stop=(oi == NH - 1),
                        )
                cols = slice(c * CH, (c + 1) * CH)
                nc.scalar.activation(expm[:, cols], pe2[:], AF.Exp)
                nc.vector.tensor_mul(expm[:, cols], expm[:, cols], mask_sb[:, cols])
                nc.vector.tensor_copy(w_bf16[:, cols], expm[:, cols])
                for st in range(CH):
                    j = c * CH + st
                    nc.tensor.matmul(
                        ctx_psum[:],
                        w_bf16[:, j : j + 1],
                        enc_bf[st][:],
                        start=(j == 0),
                        stop=(j == NST - 1),
                        skip_group_check=True,
                    )

            # ---------------- main loop, software-pipelined ---------------
            for c in range(NCHUNK):
                enc_bf, encT = a_tiles.pop(c)
                if c + 2 < NCHUNK:
                    a_tiles[c + 2] = stage_a(c + 2)
                stage_b(c, enc_bf, encT)

            # ---------------- tail ----------------
            zrow = singles.tile([P, 1], F32)
            nc.vector.reduce_sum(zrow[:], expm[:], axis=mybir.AxisListType.X)
            z_psum = pe_pool.tile([1, 1], F32, tag="pe")
            nc.tensor.matmul(z_psum[:], ones_col[:], zrow[:], start=True, stop=True)
            nc.vector.tensor_copy(ar_sb[0:1, 0:H], ctx_psum[:])
            nc.vector.tensor_copy(ar_sb[0:1, H : H + 1], z_psum[:])
            nc.sync.dma_start(ar_in[:], ar_sb[:])
            nc.gpsimd.collective_compute(
                COLLECTIVE,
                ALU.bypass if COLLECTIVE == "AllGather" else ALU.add,
                replica_groups=[list(range(NCORES))],
                ins=[ar_in[:]],
                outs=[ag_out[:]],
            )
            # transpose expm for the contiguous store; overlaps the collective
            pa = pt_pool.tile([NST, P], F32, tag="pt")
            nc.tensor.transpose(pa[:], expm[:], id_f[:])
            expm_T = singles.tile([NST, P], F32)
            nc.vector.tensor_copy(expm_T[:], pa[:])

            rz = singles.tile([1, 1], F32)
            ctx_sb = singles.tile([1, H], F32)
            if COLLECTIVE == "AllGather":
                g_sb = singles.tile([NCORES, AR_N], F32)
                nc.sync.dma_start(g_sb[:], ag_out[:])
                # reduce the 8 ranks' partials on PE
                gy = pe_pool.tile([1, H], F32, tag="pe")
                nc.tensor.matmul(
                    gy[:], ones8[:], g_sb[:, 0:H], start=True, stop=True
                )
                gz = pt_pool.tile([1, 1], F32, tag="pt")
                nc.tensor.matmul(
                    gz[:], ones8[:], g_sb[:, H : H + 1], start=True, stop=True
                )
                nc.vector.reciprocal(rz[:], gz[:])
                nc.vector.tensor_scalar_mul(ctx_sb[:], gy[:], rz[:])
            else:
                g_sb = singles.tile([1, AR_N], F32)
                nc.sync.dma_start(g_sb[:], ag_out[:])
                nc.vector.reciprocal(rz[:], g_sb[0:1, H : H + 1])
                nc.vector.tensor_scalar_mul(ctx_sb[:], g_sb[0:1, 0:H], rz[:])
            nc.sync.dma_start(ctx_out[:], ctx_sb[:])
            # w = expm/Z: broadcast 1/Z to 64 partitions, scale, store
            b_psum = pe_pool.tile([NST, 1], F32, tag="pe")
            nc.tensor.matmul(
                b_psum[:], ones_row[0:1, 0:NST], rz[:], start=True, stop=True
            )
            rz_col = singles.tile([NST, 1], F32)
            nc.vector.tensor_copy(rz_col[:], b_psum[:])
            w_T = singles.tile([NST, P], F32)
            nc.vector.tensor_scalar_mul(w_T[:], expm_T[:], rz_col[:])
            nc.sync.dma_start(attn_out[:].rearrange("(j p) -> j p", p=P), w_T[:])

    return _fix_wait_overflow(nc)


_NC = None


def _get_nc():
    global _NC
    if _NC is None:
        _NC = build_nc()
    return _NC


def make_in_maps(encoder_output, decoder_hidden, attention_mask, Wenc_w, Wdec_w, va_w):
    enc = np.ascontiguousarray(np.asarray(encoder_output, dtype=np.float32))[0]
    dec = np.ascontiguousarray(np.asarray(decoder_hidden, dtype=np.float32))
    mask_f = np.asarray(attention_mask).astype(np.float32)
    wenc = np.ascontiguousarray(np.asarray(Wenc_w, dtype=np.float32))
    wdec = np.ascontiguousarray(np.asarray(Wdec_w, dtype=np.float32))
    va = np.ascontiguousarray(np.asarray(va_w, dtype=np.float32))

    cid_f = np.zeros((P, 2 * P), dtype=np.float32)
    cid_f[:, :P] = np.eye(P, dtype=np.float32)
    cid_f[:, P:] = 1.0
    cid_bf = np.eye(P, dtype=ml_dtypes.bfloat16)

    in_maps = []
    for r in range(NCORES):
        sl = slice(r * SLOC, (r + 1) * SLOC)
        in_maps.append(
            {
                "enc": np.ascontiguousarray(enc[sl]),
                "dec": dec,
                "msk": np.ascontiguousarray(mask_f[sl]),
                "wenc": wenc,
                "wdec": wdec,
                "va": va,
                "cid_f": cid_f,
                "cid_bf": cid_bf,
            }
        )
    return in_maps


def kernel(encoder_output, decoder_hidden, attention_mask, Wenc_w, Wdec_w, va_w):
    in_maps = make_in_maps(
        encoder_output, decoder_hidden, attention_mask, Wenc_w, Wdec_w, va_w
    )
    res = run_bass_kernel_spmd(_get_nc(), in_maps, list(range(NCORES)))

    attn = np.concatenate(
        [np.asarray(res.results[r]["attn_out"]).reshape(-1) for r in range(NCORES)]
    ).reshape(1, S, 1)
    ctx = np.asarray(res.results[0]["ctx_out"]).reshape(1, 1, H)
    return ctx, attn


# revision 43
# speedup vs baseline: 1.0330x; 1.0330x over previous
"""Distributed Bass kernel for additive (Bahdanau-style) attention on 8 TRN2
NeuronCores.

Math (reference):
    temp_enc = enc @ Wenc^T                    [1,S,H]
    temp_dec = dec @ Wdec^T                    [1,H]
    x        = tanh(temp_dec + temp_enc)
    e        = x @ va^T                        [1,S,1]
    w        = softmax(mask ? e : -inf)        [1,S,1]
    ctx      = w^T @ enc                       [1,1,H]

Strategy: shard S across 8 cores (8192 rows each). Each core:
  - streams its enc shard from HBM once (f32), casts to bf16, transposes
    s<->h on TensorE (is_transpose matmuls; the DMA-xbar path costs ~1.2us
    of HWDGE descriptor-gen per 128x128 block on this runtime, measured),
  - temp_enc^T = WencT @ encT on TensorE (bf16, f32 accum in PSUM),
  - tanh+bias fused on ScalarE (bias = temp_dec as per-partition scalar,
    since the layout is transposed), output bf16,
  - energies e = va^T x on TensorE with the tanh blocks stationary, so each
    energy column lands directly in the softmax-friendly [128, 64] layout,
  - exp WITHOUT max subtraction (|e| <= ||va||_1 ~ 18, safely in f32),
    masked, then partial context = sum_s exp(e_s) * enc_s accumulated in
    PSUM across all 64 s-tiles, overlapped with the next chunk's matmuls,
  - ONE 2KB AllReduce of [partial_ctx(512) | partial_Z(1)],
  - normalize: ctx = y/Z, w_s = exp(e_s)/Z; the attention weights are
    transposed on TensorE (overlapping the collective) so the store is one
    contiguous DMA.

All host<->device tensors stay contiguous; every partition-spread (dec, va,
temp_dec, mask, attention-weight store) goes through tiny TensorE transposes
or K=1 broadcast matmuls instead of 4-byte-strided DMA descriptors.

Exactness vs reference: softmax(e) is invariant to the max subtraction up to
f32 rounding; bf16 is only used for matmul operands (f32 accumulation).
"""

import numpy as np

import ml_dtypes

import concourse.bass as bass
import concourse.mybir as mybir
import concourse.tile as tile
from concourse.bass_utils import run_bass_kernel_spmd

COLLECTIVE = "AllReduce"  # or "AllGather" (+ on-chip rank reduction)

NCORES = 8
H = 512
S = 65536
SLOC = S // NCORES  # 8192
P = 128
NST = SLOC // P  # 64 s-tiles of 128
CH = 4  # s-tiles per chunk (chunk = 512 sequence positions)
NCHUNK = NST // CH  # 16
NH = H // P  # 4 h-tiles
AR_N = 520  # 512 ctx + 1 Z + 7 pad (32B-aligned total)

F32 = mybir.dt.float32
BF16 = mybir.dt.bfloat16
AF = mybir.ActivationFunctionType
ALU = mybir.AluOpType

# ---------------------------------------------------------------------------
# walrus on this container rejects >1 sync-wait per instruction; hoist the
# excess onto same-engine NOPs placed immediately before the instruction.
_DEFAULT_LIMIT = 1


def _fix_wait_overflow(nc):
    eng_map = {
        mybir.EngineType.PE: nc.tensor,
        mybir.EngineType.DVE: nc.vector,
        mybir.EngineType.Activation: nc.scalar,
        mybir.EngineType.Pool: nc.gpsimd,
        mybir.EngineType.SP: nc.sync,
    }
    for bb in nc.main_func.blocks:
        insts = bb.instructions
        i = 0
        while i < len(insts):
            ins = insts[i]
            limit = _DEFAULT_LIMIT
            si = ins.sync_info
            waits = list(si.on_wait) if si and si.on_wait else []
            if len(waits) > limit:
                keep, extra = waits[:limit], waits[limit:]
                eng = eng_map[ins.engine]
                nops = []
                for j in range(len(extra)):
                    nop_ins = eng.nop(nofuse=True).ins
                    nop_ins.sync_info = type(si)(on_wait=[extra[j]], on_update=[])
                    for b2 in nc.main_func.blocks:
                        if nop_ins in b2.instructions:
                            b2.instructions.remove(nop_ins)
                            break
                    nops.append(nop_ins)
                ins.sync_info = type(si)(on_wait=keep, on_update=si.on_update)
                insts[i:i] = nops
                i += len(nops)
            i += 1
    return nc


# ---------------------------------------------------------------------------
def build_nc():
    nc = bass.Bass()

    enc = nc.declare_dram_parameter("enc", [SLOC, H], F32, isOutput=False)
    dec = nc.declare_dram_parameter("dec", [1, H], F32, isOutput=False)
    msk = nc.declare_dram_parameter("msk", [SLOC], F32, isOutput=False)
    wenc = nc.declare_dram_parameter("wenc", [H, H], F32, isOutput=False)
    wdec = nc.declare_dram_parameter("wdec", [H, H], F32, isOutput=False)
    va = nc.declare_dram_parameter("va", [1, H], F32, isOutput=False)
    cid_f = nc.declare_dram_parameter("cid_f", [P, 2 * P], F32, isOutput=False)
    cid_bf = nc.declare_dram_parameter("cid_bf", [P, P], BF16, isOutput=False)
    attn_out = nc.declare_dram_parameter("attn_out", [SLOC], F32, isOutput=True)
    ctx_out = nc.declare_dram_parameter("ctx_out", [1, H], F32, isOutput=True)

    ar_in = nc.dram_tensor("ar_in", [1, AR_N], F32)
    if COLLECTIVE == "AllGather":
        ag_out = nc.dram_tensor("ag_out", [NCORES, AR_N], F32, addr_space="Shared")
    else:
        ag_out = nc.dram_tensor("ag_out", [1, AR_N], F32, addr_space="Shared")

    with tile.TileContext(nc) as tc:
        with (
            tc.tile_pool(name="singles", bufs=1) as singles,
            tc.tile_pool(name="wsetup", bufs=1) as wsetup,
            tc.tile_pool(name="io", bufs=12) as io,
            tc.tile_pool(name="pt", bufs=4, space="PSUM") as pt_pool,
            tc.tile_pool(name="psum1", bufs=2, space="PSUM") as psum1,
            tc.tile_pool(name="pe", bufs=1, space="PSUM") as pe_pool,
            tc.tile_pool(name="psum_ctx", bufs=1, space="PSUM") as psum_ctx,
        ):
            # ---------------- prefetch first enc chunks + small inputs ------
            enc_pre = {}

            def load_chunk(c):
                s0 = c * CH * P
                tiles = []
                for st in range(CH):
                    t = io.tile([P, H], F32, tag="enc_f32")
                    nc.sync.dma_start(t[:], enc[s0 + st * P : s0 + (st + 1) * P, :])
                    tiles.append(t)
                return tiles

            # ---------------- small constants (DMA'd, not generated) -------
            cf = singles.tile([P, 2 * P], F32)
            nc.sync.dma_start(cf[:], cid_f[:])
            id_bf = singles.tile([P, P], BF16)
            nc.sync.dma_start(id_bf[:], cid_bf[:])

            enc_pre[0] = load_chunk(0)
            # Wenc load goes out right after chunk 0 so mm1 can start early
            we_f32 = []
            for oi in range(NH):
                t = wsetup.tile([P, H], F32, tag=f"wef32_{oi}")
                nc.sync.dma_start(t[:], wenc[oi * P : (oi + 1) * P, :])
                we_f32.append(t)
            enc_pre[1] = load_chunk(1)
            dec_nat = singles.tile([1, H], F32)
            va_nat = singles.tile([1, H], F32)
            nc.sync.dma_start(dec_nat[:], dec[:])
            nc.sync.dma_start(va_nat[:], va[:])
            mask_nat = singles.tile([NST, P], F32)
            nc.sync.dma_start(mask_nat[:], msk[:].rearrange("(j p) -> j p", p=P))

            id_f = cf[:, 0:P]
            one1 = cf[0:1, P : P + 1]
            ones_col = cf[:, P : P + 1]
            ones_row = cf[0:1, P : 2 * P]
            ones8 = cf[0:NCORES, P : P + 1]

            # ---------------- stage A: cast + transpose one chunk ----------
            def stage_a(c):
                enc_f32 = enc_pre.pop(c) if c in enc_pre else load_chunk(c)
                enc_bf = []
                for st in range(CH):
                    t = io.tile([P, H], BF16, tag="enc_bf")
                    nc.vector.tensor_copy(t[:], enc_f32[st][:])
                    enc_bf.append(t)
                encT = []
                for hi in range(NH):
                    ptt = pt_pool.tile([P, CH * P], BF16, tag="pt")
                    for st in range(CH):
                        nc.tensor.transpose(
                            ptt[:, st * P : (st + 1) * P],
                            enc_bf[st][:, hi * P : (hi + 1) * P],
                            id_bf[:],
                        )
                    t = io.tile([P, CH * P], BF16, tag="encT")
                    nc.vector.tensor_copy(t[:], ptt[:])
                    encT.append(t)
                return enc_bf, encT

            a_tiles = {0: stage_a(0), 1: stage_a(1)}

            # ---------------- remaining setup ----------------
            # Wenc -> WencT bf16 via PE transposes
            wencT = []
            for hi in range(NH):
                pw = pt_pool.tile([P, H], F32, tag="pt")
                for oi in range(NH):
                    nc.tensor.transpose(
                        pw[:, oi * P : (oi + 1) * P],
                        we_f32[oi][:, hi * P : (hi + 1) * P],
                        id_f[:],
                    )
                t = wsetup.tile([P, H], BF16, tag=f"weT_{hi}")
                nc.vector.tensor_copy(t[:], pw[:])
                wencT.append(t)

            # va columns [128, NH] bf16 via K=1 broadcast matmuls
            pc = pe_pool.tile([P, NH], F32, tag="pe")
            for hi in range(NH):
                nc.tensor.matmul(
                    pc[:, hi : hi + 1],
                    va_nat[0:1, hi * P : (hi + 1) * P],
                    one1[:],
                    start=True,
                    stop=True,
                )
            va_col = singles.tile([P, NH], BF16)
            nc.vector.tensor_copy(va_col[:], pc[:])

            # Wdec natural + temp_dec on DVE
            wd_f32 = []
            for oi in range(NH):
                t = wsetup.tile([P, H], F32, tag=f"wdf32_{oi}")
                nc.sync.dma_start(t[:], wdec[oi * P : (oi + 1) * P, :])
                wd_f32.append(t)
            pdb = pe_pool.tile([P, H], F32, tag="pe")
            nc.tensor.matmul(pdb[:], ones_row[:], dec_nat[:], start=True, stop=True)
            dec_bcast = singles.tile([P, H], F32)
            nc.vector.tensor_copy(dec_bcast[:], pdb[:])
            td_col = singles.tile([P, NH], F32)
            tt_scratch = singles.tile([P, H], F32)
            for oi in range(NH):
                nc.vector.tensor_mul(tt_scratch[:], wd_f32[oi][:], dec_bcast[:])
                nc.vector.reduce_sum(
                    td_col[:, oi : oi + 1],
                    tt_scratch[:],
                    axis=mybir.AxisListType.X,
                )

            # mask: PE-transpose [64,128] -> [128,64]
            pm = pt_pool.tile([P, NST], F32, tag="pt")
            nc.tensor.transpose(pm[:], mask_nat[:], id_f[:NST, :NST])
            mask_sb = singles.tile([P, NST], F32)
            nc.vector.tensor_copy(mask_sb[:], pm[:])

            # persistent softmax state
            expm = singles.tile([P, NST], F32)  # masked exp(e)
            w_bf16 = singles.tile([P, NST], BF16)  # bf16 copy for ctx matmul
            ctx_psum = psum_ctx.tile([1, H], F32)
            ar_sb = singles.tile([1, AR_N], F32)
            nc.vector.memset(ar_sb[:], 0.0)

            # ---------------- stage B: matmuls + softmax pieces ------------
            def stage_b(c, enc_bf, encT):
                x_bf = []
                for oi in range(NH):
                    p1 = psum1.tile([P, CH * P], F32, tag="psum1")
                    for hi in range(NH):
                        nc.tensor.matmul(
                            p1[:],
                            wencT[hi][:, oi * P : (oi + 1) * P],
                            encT[hi][:],
                            start=(hi == 0),
                            stop=(hi == NH - 1),
                        )
                    xt = io.tile([P, CH * P], BF16, tag="x_bf")
                    nc.scalar.activation(
                        xt[:], p1[:], AF.Tanh, bias=td_col[:, oi : oi + 1]
                    )
                    x_bf.append(xt)
                # energies into [128, CH] columns (x blocks stationary)
                pe2 = pe_pool.tile([P, CH], F32, tag="pe")
                for st in range(CH):
                    for oi in range(NH):
                        nc.tensor.matmul(
                            pe2[:, st : st + 1],
                            x_bf[oi][:, st * P : (st + 1) * P],
                            va_col[:, oi : oi + 1],
                            start=(oi == 0),
                            stop=(oi == NH - 1),
                        )
                cols = slice(c * CH, (c + 1) * CH)
                nc.scalar.activation(expm[:, cols], pe2[:], AF.Exp)
                nc.vector.tensor_mul(expm[:, cols], expm[:, cols], mask_sb[:, cols])
                nc.vector.tensor_copy(w_bf16[:, cols], expm[:, cols])
                for st in range(CH):
                    j = c * CH + st
                    nc.tensor.matmul(
                        ctx_psum[:],
                        w_bf16[:, j : j + 1],
                        enc_bf[st][:],
                        start=(j == 0),
                        stop=(j == NST - 1),
                        skip_group_check=True,
                    )

            # ---------------- main loop, software-pipelined ---------------
            for c in range(NCHUNK):
                enc_bf, encT = a_tiles.pop(c)
                if c + 2 < NCHUNK:
                    a_tiles[c + 2] = stage_a(c + 2)
                stage_b(c, enc_bf, encT)

            # ---------------- tail ----------------
            zrow = singles.tile([P, 1], F32)
            nc.vector.reduce_sum(zrow[:], expm[:], axis=mybir.AxisListType.X)
            z_psum = pe_pool.tile([1, 1], F32, tag="pe")
            nc.tensor.matmul(z_psum[:], ones_col[:], zrow[:], start=True, stop=True)
            nc.vector.tensor_copy(ar_sb[0:1, 0:H], ctx_psum[:])
            nc.vector.tensor_copy(ar_sb[0:1, H : H + 1], z_psum[:])
            nc.sync.dma_start(ar_in[:], ar_sb[:])
            nc.gpsimd.collective_compute(
                COLLECTIVE,
                ALU.bypass if COLLECTIVE == "AllGather" else ALU.add,
                replica_groups=[list(range(NCORES))],
                ins=[ar_in[:]],
                outs=[ag_out[:]],
            )
            # transpose expm for the contiguous store; overlaps the collective
            pa = pt_pool.tile([NST, P], F32, tag="pt")
            nc.tensor.transpose(pa[:], expm[:], id_f[:])
            expm_T = singles.tile([NST, P], F32)
            nc.vector.tensor_copy(expm_T[:], pa[:])

            rz = singles.tile([1, 1], F32)
            ctx_sb = singles.tile([1, H], F32)
            if COLLECTIVE == "AllGather":
                g_sb = singles.tile([NCORES, AR_N], F32)
                nc.sync.dma_start(g_sb[:], ag_out[:])
                # reduce the 8 ranks' partials on PE
                gy = pe_pool.tile([1, H], F32, tag="pe")
                nc.tensor.matmul(
                    gy[:], ones8[:], g_sb[:, 0:H], start=True, stop=True
                )
                gz = pt_pool.tile([1, 1], F32, tag="pt")
                nc.tensor.matmul(
                    gz[:], ones8[:], g_sb[:, H : H + 1], start=True, stop=True
                )
                nc.vector.reciprocal(rz[:], gz[:])
                nc.vector.tensor_scalar_mul(ctx_sb[:], gy[:], rz[:])
            else:
                g_sb = singles.tile([1, AR_N], F32)
                nc.sync.dma_start(g_sb[:], ag_out[:])
                nc.vector.reciprocal(rz[:], g_sb[0:1, H : H + 1])
                nc.vector.tensor_scalar_mul(ctx_sb[:], g_sb[0:1, 0:H], rz[:])
            nc.sync.dma_start(ctx_out[:], ctx_sb[:])
            # w = expm/Z: broadcast 1/Z to 64 partitions, scale, store
            b_psum = pe_pool.tile([NST, 1], F32, tag="pe")
            nc.tensor.matmul(
                b_psum[:], ones_row[0:1, 0:NST], rz[:], start=True, stop=True
            )
            rz_col = singles.tile([NST, 1], F32)
            nc.vector.tensor_copy(rz_col[:], b_psum[:])
            w_T = singles.tile([NST, P], F32)
            nc.vector.tensor_scalar_mul(w_T[:], expm_T[:], rz_col[:])
            nc.sync.dma_start(attn_out[:].rearrange("(j p) -> j p", p=P), w_T[:])

    return _fix_wait_overflow(nc)


_NC = None


def _get_nc():
    global _NC
    if _NC is None:
        _NC = build_nc()
    return _NC


def make_in_maps(encoder_output, decoder_hidden, attention_mask, Wenc_w, Wdec_w, va_w):
    enc = np.ascontiguousarray(np.asarray(encoder_output, dtype=np.float32))[0]
    dec = np.ascontiguousarray(np.asarray(decoder_hidden, dtype=np.float32))
    mask_f = np.asarray(attention_mask).astype(np.float32)
    wenc = np.ascontiguousarray(np.asarray(Wenc_w, dtype=np.float32))
    wdec = np.ascontiguousarray(np.asarray(Wdec_w, dtype=np.float32))
    va = np.ascontiguousarray(np.asarray(va_w, dtype=np.float32))

    cid_f = np.zeros((P, 2 * P), dtype=np.float32)
    cid_f[:, :P] = np.eye(P, dtype=np.float32)
    cid_f[:, P:] = 1.0
    cid_bf = np.eye(P, dtype=ml_dtypes.bfloat16)

    in_maps = []
    for r in range(NCORES):
        sl = slice(r * SLOC, (r + 1) * SLOC)
        in_maps.append(
            {
                "enc": np.ascontiguousarray(enc[sl]),
                "dec": dec,
                "msk": np.ascontiguousarray(mask_f[sl]),
                "wenc": wenc,
                "wdec": wdec,
                "va": va,
                "cid_f": cid_f,
                "cid_bf": cid_bf,
            }
        )
    return in_maps


def kernel(encoder_output, decoder_hidden, attention_mask, Wenc_w, Wdec_w, va_w):
    in_maps = make_in_maps(
        encoder_output, decoder_hidden, attention_mask, Wenc_w, Wdec_w, va_w
    )
    res = run_bass_kernel_spmd(_get_nc(), in_maps, list(range(NCORES)))

    attn = np.concatenate(
        [np.asarray(res.results[r]["attn_out"]).reshape(-1) for r in range(NCORES)]
    ).reshape(1, S, 1)
    ctx = np.asarray(res.results[0]["ctx_out"]).reshape(1, 1, H)
    return ctx, attn


# revision 44
# speedup vs baseline: 1.0395x; 1.0063x over previous
"""Distributed Bass kernel for additive (Bahdanau-style) attention on 8 TRN2
NeuronCores.

Math (reference):
    temp_enc = enc @ Wenc^T                    [1,S,H]
    temp_dec = dec @ Wdec^T                    [1,H]
    x        = tanh(temp_dec + temp_enc)
    e        = x @ va^T                        [1,S,1]
    w        = softmax(mask ? e : -inf)        [1,S,1]
    ctx      = w^T @ enc                       [1,1,H]

Strategy: shard S across 8 cores (8192 rows each). Each core:
  - streams its enc shard from HBM once (f32), casts to bf16, transposes
    s<->h on TensorE (is_transpose matmuls; the DMA-xbar path costs ~1.2us
    of HWDGE descriptor-gen per 128x128 block on this runtime, measured),
  - temp_enc^T = WencT @ encT on TensorE (bf16, f32 accum in PSUM),
  - tanh+bias fused on ScalarE (bias = temp_dec as per-partition scalar,
    since the layout is transposed), output bf16,
  - energies e = va^T x on TensorE with the tanh blocks stationary, so each
    energy column lands directly in the softmax-friendly [128, 64] layout,
  - exp WITHOUT max subtraction (|e| <= ||va||_1 ~ 18, safely in f32),
    masked, then partial context = sum_s exp(e_s) * enc_s accumulated in
    PSUM across all 64 s-tiles, overlapped with the next chunk's matmuls,
  - ONE 2KB AllReduce of [partial_ctx(512) | partial_Z(1)],
  - normalize: ctx = y/Z, w_s = exp(e_s)/Z; the attention weights are
    transposed on TensorE (overlapping the collective) so the store is one
    contiguous DMA.

All host<->device tensors stay contiguous; every partition-spread (dec, va,
temp_dec, mask, attention-weight store) goes through tiny TensorE transposes
or K=1 broadcast matmuls instead of 4-byte-strided DMA descriptors.

Exactness vs reference: softmax(e) is invariant to the max subtraction up to
f32 rounding; bf16 is only used for matmul operands (f32 accumulation).
"""

import numpy as np

import ml_dtypes

import concourse.bass as bass
import concourse.mybir as mybir
import concourse.tile as tile
from concourse.bass_utils import run_bass_kernel_spmd

COLLECTIVE = "AllReduce"  # or "AllGather" (+ on-chip rank reduction)

NCORES = 8
H = 512
S = 65536
SLOC = S // NCORES  # 8192
P = 128
NST = SLOC // P  # 64 s-tiles of 128
CH = 4  # s-tiles per chunk (chunk = 512 sequence positions)
NCHUNK = NST // CH  # 16
NH = H // P  # 4 h-tiles
AR_N = 520  # 512 ctx + 1 Z + 7 pad (32B-aligned total)

F32 = mybir.dt.float32
BF16 = mybir.dt.bfloat16
AF = mybir.ActivationFunctionType
ALU = mybir.AluOpType

# ---------------------------------------------------------------------------
# walrus on this container rejects >1 sync-wait per instruction; hoist the
# excess onto same-engine NOPs placed immediately before the instruction.
_DEFAULT_LIMIT = 1


def _fix_wait_overflow(nc):
    eng_map = {
        mybir.EngineType.PE: nc.tensor,
        mybir.EngineType.DVE: nc.vector,
        mybir.EngineType.Activation: nc.scalar,
        mybir.EngineType.Pool: nc.gpsimd,
        mybir.EngineType.SP: nc.sync,
    }
    for bb in nc.main_func.blocks:
        insts = bb.instructions
        i = 0
        while i < len(insts):
            ins = insts[i]
            limit = _DEFAULT_LIMIT
            si = ins.sync_info
            waits = list(si.on_wait) if si and si.on_wait else []
            if len(waits) > limit:
                keep, extra = waits[:limit], waits[limit:]
                eng = eng_map[ins.engine]
                nops = []
                for j in range(len(extra)):
                    nop_ins = eng.nop(nofuse=True).ins
                    nop_ins.sync_info = type(si)(on_wait=[extra[j]], on_update=[])
                    for b2 in nc.main_func.blocks:
                        if nop_ins in b2.instructions:
                            b2.instructions.remove(nop_ins)
                            break
                    nops.append(nop_ins)
                ins.sync_info = type(si)(on_wait=keep, on_update=si.on_update)
                insts[i:i] = nops
                i += len(nops)
            i += 1
    return nc


# ---------------------------------------------------------------------------
def build_nc():
    nc = bass.Bass()

    enc = nc.declare_dram_parameter("enc", [SLOC, H], F32, isOutput=False)
    dec = nc.declare_dram_parameter("dec", [1, H], F32, isOutput=False)
    msk = nc.declare_dram_parameter("msk", [SLOC], F32, isOutput=False)
    wenc = nc.declare_dram_parameter("wenc", [H, H], F32, isOutput=False)
    wdec = nc.declare_dram_parameter("wdec", [H, H], F32, isOutput=False)
    va = nc.declare_dram_parameter("va", [1, H], F32, isOutput=False)
    cid_f = nc.declare_dram_parameter("cid_f", [P, 2 * P], F32, isOutput=False)
    cid_bf = nc.declare_dram_parameter("cid_bf", [P, P], BF16, isOutput=False)
    attn_out = nc.declare_dram_parameter("attn_out", [SLOC], F32, isOutput=True)
    ctx_out = nc.declare_dram_parameter("ctx_out", [1, H], F32, isOutput=True)

    ar_in = nc.dram_tensor("ar_in", [1, AR_N], F32)
    if COLLECTIVE == "AllGather":
        ag_out = nc.dram_tensor("ag_out", [NCORES, AR_N], F32, addr_space="Shared")
    else:
        ag_out = nc.dram_tensor("ag_out", [1, AR_N], F32, addr_space="Shared")

    with tile.TileContext(nc) as tc:
        with (
            tc.tile_pool(name="singles", bufs=1) as singles,
            tc.tile_pool(name="wsetup", bufs=1) as wsetup,
            tc.tile_pool(name="io", bufs=12) as io,
            tc.tile_pool(name="pt", bufs=4, space="PSUM") as pt_pool,
            tc.tile_pool(name="psum1", bufs=2, space="PSUM") as psum1,
            tc.tile_pool(name="pe", bufs=1, space="PSUM") as pe_pool,
            tc.tile_pool(name="psum_ctx", bufs=1, space="PSUM") as psum_ctx,
        ):
            # ---------------- prefetch first enc chunks + small inputs ------
            enc_pre = {}

            def load_chunk(c):
                s0 = c * CH * P
                tiles = []
                for st in range(CH):
                    t = io.tile([P, H], F32, tag="enc_f32")
                    nc.sync.dma_start(t[:], enc[s0 + st * P : s0 + (st + 1) * P, :])
                    tiles.append(t)
                return tiles

            # ---------------- small constants (DMA'd, not generated) -------
            cf = singles.tile([P, 2 * P], F32)
            nc.scalar.dma_start(cf[:], cid_f[:])
            id_bf = singles.tile([P, P], BF16)
            nc.scalar.dma_start(id_bf[:], cid_bf[:])

            enc_pre[0] = load_chunk(0)
            # Wenc load goes out right after chunk 0 so mm1 can start early
            we_f32 = []
            for oi in range(NH):
                t = wsetup.tile([P, H], F32, tag=f"wef32_{oi}")
                nc.scalar.dma_start(t[:], wenc[oi * P : (oi + 1) * P, :])
                we_f32.append(t)
            enc_pre[1] = load_chunk(1)
            dec_nat = singles.tile([1, H], F32)
            va_nat = singles.tile([1, H], F32)
            nc.scalar.dma_start(dec_nat[:], dec[:])
            nc.scalar.dma_start(va_nat[:], va[:])
            mask_nat = singles.tile([NST, P], F32)
            nc.scalar.dma_start(mask_nat[:], msk[:].rearrange("(j p) -> j p", p=P))

            id_f = cf[:, 0:P]
            one1 = cf[0:1, P : P + 1]
            ones_col = cf[:, P : P + 1]
            ones_row = cf[0:1, P : 2 * P]
            ones8 = cf[0:NCORES, P : P + 1]

            # ---------------- stage A: cast + transpose one chunk ----------
            def stage_a(c):
                enc_f32 = enc_pre.pop(c) if c in enc_pre else load_chunk(c)
                enc_bf = []
                for st in range(CH):
                    t = io.tile([P, H], BF16, tag="enc_bf")
                    nc.vector.tensor_copy(t[:], enc_f32[st][:])
                    enc_bf.append(t)
                encT = []
                for hi in range(NH):
                    ptt = pt_pool.tile([P, CH * P], BF16, tag="pt")
                    for st in range(CH):
                        nc.tensor.transpose(
                            ptt[:, st * P : (st + 1) * P],
                            enc_bf[st][:, hi * P : (hi + 1) * P],
                            id_bf[:],
                        )
                    t = io.tile([P, CH * P], BF16, tag="encT")
                    nc.vector.tensor_copy(t[:], ptt[:])
                    encT.append(t)
                return enc_bf, encT

            a_tiles = {0: stage_a(0), 1: stage_a(1)}

            # ---------------- remaining setup ----------------
            # Wenc -> WencT bf16 via PE transposes
            wencT = []
            for hi in range(NH):
                pw = pt_pool.tile([P, H], F32, tag="pt")
                for oi in range(NH):
                    nc.tensor.transpose(
                        pw[:, oi * P : (oi + 1) * P],
                        we_f32[oi][:, hi * P : (hi + 1) * P],
                        id_f[:],
                    )
                t = wsetup.tile([P, H], BF16, tag=f"weT_{hi}")
                nc.vector.tensor_copy(t[:], pw[:])
                wencT.append(t)

            # va columns [128, NH] bf16 via K=1 broadcast matmuls
            pc = pe_pool.tile([P, NH], F32, tag="pe")
            for hi in range(NH):
                nc.tensor.matmul(
                    pc[:, hi : hi + 1],
                    va_nat[0:1, hi * P : (hi + 1) * P],
                    one1[:],
                    start=True,
                    stop=True,
                )
            va_col = singles.tile([P, NH], BF16)
            nc.vector.tensor_copy(va_col[:], pc[:])

            # Wdec natural + temp_dec on DVE
            wd_f32 = []
            for oi in range(NH):
                t = wsetup.tile([P, H], F32, tag=f"wdf32_{oi}")
                nc.scalar.dma_start(t[:], wdec[oi * P : (oi + 1) * P, :])
                wd_f32.append(t)
            pdb = pe_pool.tile([P, H], F32, tag="pe")
            nc.tensor.matmul(pdb[:], ones_row[:], dec_nat[:], start=True, stop=True)
            dec_bcast = singles.tile([P, H], F32)
            nc.vector.tensor_copy(dec_bcast[:], pdb[:])
            td_col = singles.tile([P, NH], F32)
            tt_scratch = singles.tile([P, H], F32)
            for oi in range(NH):
                nc.vector.tensor_mul(tt_scratch[:], wd_f32[oi][:], dec_bcast[:])
                nc.vector.reduce_sum(
                    td_col[:, oi : oi + 1],
                    tt_scratch[:],
                    axis=mybir.AxisListType.X,
                )

            # mask: PE-transpose [64,128] -> [128,64]
            pm = pt_pool.tile([P, NST], F32, tag="pt")
            nc.tensor.transpose(pm[:], mask_nat[:], id_f[:NST, :NST])
            mask_sb = singles.tile([P, NST], F32)
            nc.vector.tensor_copy(mask_sb[:], pm[:])

            # persistent softmax state
            expm = singles.tile([P, NST], F32)  # masked exp(e)
            w_bf16 = singles.tile([P, NST], BF16)  # bf16 copy for ctx matmul
            ctx_psum = psum_ctx.tile([1, H], F32)
            ar_sb = singles.tile([1, AR_N], F32)
            nc.vector.memset(ar_sb[:], 0.0)

            # ---------------- stage B: matmuls + softmax pieces ------------
            def stage_b(c, enc_bf, encT):
                x_bf = []
                for oi in range(NH):
                    p1 = psum1.tile([P, CH * P], F32, tag="psum1")
                    for hi in range(NH):
                        nc.tensor.matmul(
                            p1[:],
                            wencT[hi][:, oi * P : (oi + 1) * P],
                            encT[hi][:],
                            start=(hi == 0),
                            stop=(hi == NH - 1),
                        )
                    xt = io.tile([P, CH * P], BF16, tag="x_bf")
                    nc.scalar.activation(
                        xt[:], p1[:], AF.Tanh, bias=td_col[:, oi : oi + 1]
                    )
                    x_bf.append(xt)
                # energies into [128, CH] columns (x blocks stationary)
                pe2 = pe_pool.tile([P, CH], F32, tag="pe")
                for st in range(CH):
                    for oi in range(NH):
                        nc.tensor.matmul(
                            pe2[:, st : st + 1],
                            x_bf[oi][:, st * P : (st + 1) * P],
                            va_col[:, oi : oi + 1],
                            start=(oi == 0),
                            stop=(oi == NH - 1),
                        )
                cols = slice(c * CH, (c + 1) * CH)
                nc.scalar.activation(expm[:, cols], pe2[:], AF.Exp)
                nc.vector.tensor_mul(expm[:, cols], expm[:, cols], mask_sb[:, cols])
                nc.vector.tensor_copy(w_bf16[:, cols], expm[:, cols])
                for st in range(CH):
                    j = c * CH + st
                    nc.tensor.matmul(
                        ctx_psum[:],
                        w_bf16[:, j : j + 1],
                        enc_bf[st][:],
                        start=(j == 0),
                        stop=(j == NST - 1),
                        skip_group_check=True,
                    )

            # ---------------- main loop, software-pipelined ---------------
            for c in range(NCHUNK):
                enc_bf, encT = a_tiles.pop(c)
                if c + 2 < NCHUNK:
                    a_tiles[c + 2] = stage_a(c + 2)
                stage_b(c, enc_bf, encT)

            # ---------------- tail ----------------
            zrow = singles.tile([P, 1], F32)
            nc.vector.reduce_sum(zrow[:], expm[:], axis=mybir.AxisListType.X)
            z_psum = pe_pool.tile([1, 1], F32, tag="pe")
            nc.tensor.matmul(z_psum[:], ones_col[:], zrow[:], start=True, stop=True)
            nc.vector.tensor_copy(ar_sb[0:1, 0:H], ctx_psum[:])
            nc.vector.tensor_copy(ar_sb[0:1, H : H + 1], z_psum[:])
            nc.sync.dma_start(ar_in[:], ar_sb[:])
            nc.gpsimd.collective_compute(
                COLLECTIVE,
                ALU.bypass if COLLECTIVE == "AllGather" else ALU.add,
                replica_groups=[list(range(NCORES))],
                ins=[ar_in[:]],
                outs=[ag_out[:]],
            )
            # transpose expm for the contiguous store; overlaps the collective
            pa = pt_pool.tile([NST, P], F32, tag="pt")
            nc.tensor.transpose(pa[:], expm[:], id_f[:])
            expm_T = singles.tile([NST, P], F32)
            nc.vector.tensor_copy(expm_T[:], pa[:])

            rz = singles.tile([1, 1], F32)
            ctx_sb = singles.tile([1, H], F32)
            if COLLECTIVE == "AllGather":
                g_sb = singles.tile([NCORES, AR_N], F32)
                nc.sync.dma_start(g_sb[:], ag_out[:])
                # reduce the 8 ranks' partials on PE
                gy = pe_pool.tile([1, H], F32, tag="pe")
                nc.tensor.matmul(
                    gy[:], ones8[:], g_sb[:, 0:H], start=True, stop=True
                )
                gz = pt_pool.tile([1, 1], F32, tag="pt")
                nc.tensor.matmul(
                    gz[:], ones8[:], g_sb[:, H : H + 1], start=True, stop=True
                )
                nc.vector.reciprocal(rz[:], gz[:])
                nc.vector.tensor_scalar_mul(ctx_sb[:], gy[:], rz[:])
            else:
                g_sb = singles.tile([1, AR_N], F32)
                nc.sync.dma_start(g_sb[:], ag_out[:])
                nc.vector.reciprocal(rz[:], g_sb[0:1, H : H + 1])
                nc.vector.tensor_scalar_mul(ctx_sb[:], g_sb[0:1, 0:H], rz[:])
            nc.sync.dma_start(ctx_out[:], ctx_sb[:])
            # w = expm/Z: broadcast 1/Z to 64 partitions, scale, store
            b_psum = pe_pool.tile([NST, 1], F32, tag="pe")
            nc.tensor.matmul(
                b_psum[:], ones_row[0:1, 0:NST], rz[:], start=True, stop=True
            )
            rz_col = singles.tile([NST, 1], F32)
            nc.vector.tensor_copy(rz_col[:], b_psum[:])
            w_T = singles.tile([NST, P], F32)
            nc.vector.tensor_scalar_mul(w_T[:], expm_T[:], rz_col[:])
            nc.sync.dma_start(attn_out[:].rearrange("(j p) -> j p", p=P), w_T[:])

    return _fix_wait_overflow(nc)


_NC = None


def _get_nc():
    global _NC
    if _NC is None:
        _NC = build_nc()
    return _NC


def make_in_maps(encoder_output, decoder_hidden, attention_mask, Wenc_w, Wdec_w, va_w):
    enc = np.ascontiguousarray(np.asarray(encoder_output, dtype=np.float32))[0]
    dec = np.ascontiguousarray(np.asarray(decoder_hidden, dtype=np.float32))
    mask_f = np.asarray(attention_mask).astype(np.float32)
    wenc = np.ascontiguousarray(np.asarray(Wenc_w, dtype=np.float32))
    wdec = np.ascontiguousarray(np.asarray(Wdec_w, dtype=np.float32))
    va = np.ascontiguousarray(np.asarray(va_w, dtype=np.float32))

    cid_f = np.zeros((P, 2 * P), dtype=np.float32)
    cid_f[:, :P] = np.eye(P, dtype=np.float32)
    cid_f[:, P:] = 1.0
    cid_bf = np.eye(P, dtype=ml_dtypes.bfloat16)

    in_maps = []
    for r in range(NCORES):
        sl = slice(r * SLOC, (r + 1) * SLOC)
        in_maps.append(
            {
                "enc": np.ascontiguousarray(enc[sl]),
                "dec": dec,
                "msk": np.ascontiguousarray(mask_f[sl]),
                "wenc": wenc,
                "wdec": wdec,
                "va": va,
                "cid_f": cid_f,
                "cid_bf": cid_bf,
            }
        )
    return in_maps


def kernel(encoder_output, decoder_hidden, attention_mask, Wenc_w, Wdec_w, va_w):
    in_maps = make_in_maps(
        encoder_output, decoder_hidden, attention_mask, Wenc_w, Wdec_w, va_w
    )
    res = run_bass_kernel_spmd(_get_nc(), in_maps, list(range(NCORES)))

    attn = np.concatenate(
        [np.asarray(res.results[r]["attn_out"]).reshape(-1) for r in range(NCORES)]
    ).reshape(1, S, 1)
    ctx = np.asarray(res.results[0]["ctx_out"]).reshape(1, 1, H)
    return ctx, attn


# revision 46
# speedup vs baseline: 1.1363x; 1.0932x over previous
"""Distributed Bass kernel for additive (Bahdanau-style) attention on 8 TRN2
NeuronCores.

Math (reference):
    temp_enc = enc @ Wenc^T                    [1,S,H]
    temp_dec = dec @ Wdec^T                    [1,H]
    x        = tanh(temp_dec + temp_enc)
    e        = x @ va^T                        [1,S,1]
    w        = softmax(mask ? e : -inf)        [1,S,1]
    ctx      = w^T @ enc                       [1,1,H]

Strategy: shard S across 8 cores (8192 rows each). Each core:
  - streams its enc shard from HBM once (f32), casts to bf16, transposes
    s<->h on TensorE (is_transpose matmuls; the DMA-xbar path costs ~1.2us
    of HWDGE descriptor-gen per 128x128 block on this runtime, measured),
  - temp_enc^T = WencT @ encT on TensorE (bf16, f32 accum in PSUM),
  - tanh+bias fused on ScalarE (bias = temp_dec as per-partition scalar,
    since the layout is transposed), output bf16,
  - energies e = va^T x on TensorE with the tanh blocks stationary, so each
    energy column lands directly in the softmax-friendly [128, 64] layout,
  - exp WITHOUT max subtraction (|e| <= ||va||_1 ~ 18, safely in f32),
    masked, then partial context = sum_s exp(e_s) * enc_s accumulated in
    PSUM across all 64 s-tiles, overlapped with the next chunk's matmuls,
  - ONE 2KB AllReduce of [partial_ctx(512) | partial_Z(1)],
  - normalize: ctx = y/Z, w_s = exp(e_s)/Z; the attention weights are
    transposed on TensorE (overlapping the collective) so the store is one
    contiguous DMA.

All host<->device tensors stay contiguous; every partition-spread (dec, va,
temp_dec, mask, attention-weight store) goes through tiny TensorE transposes
or K=1 broadcast matmuls instead of 4-byte-strided DMA descriptors.

Exactness vs reference: softmax(e) is invariant to the max subtraction up to
f32 rounding; bf16 is only used for matmul operands (f32 accumulation).
"""

import numpy as np

import ml_dtypes

import concourse.bass as bass
import concourse.mybir as mybir
import concourse.tile as tile
from concourse.bass_utils import run_bass_kernel_spmd

COLLECTIVE = "AllReduce"  # or "AllGather" (+ on-chip rank reduction)

NCORES = 8
H = 512
S = 65536
SLOC = S // NCORES  # 8192
P = 128
NST = SLOC // P  # 64 s-tiles of 128
CH = 4  # s-tiles per chunk (chunk = 512 sequence positions)
NCHUNK = NST // CH  # 16
NH = H // P  # 4 h-tiles
AR_N = 520  # 512 ctx + 1 Z + 7 pad (32B-aligned total)

F32 = mybir.dt.float32
BF16 = mybir.dt.bfloat16
AF = mybir.ActivationFunctionType
ALU = mybir.AluOpType

# ---------------------------------------------------------------------------
# walrus on this container rejects >1 sync-wait per instruction; hoist the
# excess onto same-engine NOPs placed immediately before the instruction.
_DEFAULT_LIMIT = 1


def _fix_wait_overflow(nc):
    eng_map = {
        mybir.EngineType.PE: nc.tensor,
        mybir.EngineType.DVE: nc.vector,
        mybir.EngineType.Activation: nc.scalar,
        mybir.EngineType.Pool: nc.gpsimd,
        mybir.EngineType.SP: nc.sync,
    }
    for bb in nc.main_func.blocks:
        insts = bb.instructions
        i = 0
        while i < len(insts):
            ins = insts[i]
            limit = _DEFAULT_LIMIT
            si = ins.sync_info
            waits = list(si.on_wait) if si and si.on_wait else []
            if len(waits) > limit:
                keep, extra = waits[:limit], waits[limit:]
                eng = eng_map[ins.engine]
                nops = []
                for j in range(len(extra)):
                    nop_ins = eng.nop(nofuse=True).ins
                    nop_ins.sync_info = type(si)(on_wait=[extra[j]], on_update=[])
                    for b2 in nc.main_func.blocks:
                        if nop_ins in b2.instructions:
                            b2.instructions.remove(nop_ins)
                            break
                    nops.append(nop_ins)
                ins.sync_info = type(si)(on_wait=keep, on_update=si.on_update)
                insts[i:i] = nops
                i += len(nops)
            i += 1
    return nc


# ---------------------------------------------------------------------------
def build_nc():
    nc = bass.Bass()

    enc = nc.declare_dram_parameter("enc", [SLOC, H], F32, isOutput=False)
    dec = nc.declare_dram_parameter("dec", [1, H], F32, isOutput=False)
    msk = nc.declare_dram_parameter("msk", [SLOC], F32, isOutput=False)
    wenc = nc.declare_dram_parameter("wenc", [H, H], F32, isOutput=False)
    wdec = nc.declare_dram_parameter("wdec", [H, H], F32, isOutput=False)
    va = nc.declare_dram_parameter("va", [1, H], F32, isOutput=False)
    cid_f = nc.declare_dram_parameter("cid_f", [P, 2 * P], F32, isOutput=False)
    cid_bf = nc.declare_dram_parameter("cid_bf", [P, P], BF16, isOutput=False)
    attn_out = nc.declare_dram_parameter("attn_out", [SLOC], F32, isOutput=True)
    ctx_out = nc.declare_dram_parameter("ctx_out", [1, H], F32, isOutput=True)

    ar_in = nc.dram_tensor("ar_in", [1, AR_N], F32)
    if COLLECTIVE == "AllGather":
        ag_out = nc.dram_tensor("ag_out", [NCORES, AR_N], F32, addr_space="Shared")
    else:
        ag_out = nc.dram_tensor("ag_out", [1, AR_N], F32, addr_space="Shared")

    with tile.TileContext(nc) as tc:
        with (
            tc.tile_pool(name="singles", bufs=1) as singles,
            tc.tile_pool(name="wsetup", bufs=1) as wsetup,
            tc.tile_pool(name="io", bufs=12) as io,
            tc.tile_pool(name="pt", bufs=4, space="PSUM") as pt_pool,
            tc.tile_pool(name="psum1", bufs=2, space="PSUM") as psum1,
            tc.tile_pool(name="pe", bufs=1, space="PSUM") as pe_pool,
            tc.tile_pool(name="psum_ctx", bufs=1, space="PSUM") as psum_ctx,
        ):
            # ---------------- prefetch first enc chunks + small inputs ------
            enc_pre = {}

            def load_chunk(c):
                s0 = c * CH * P
                tiles = []
                for st in range(CH):
                    t = io.tile([P, H], F32, tag="enc_f32")
                    nc.sync.dma_start(t[:], enc[s0 + st * P : s0 + (st + 1) * P, :])
                    tiles.append(t)
                return tiles

            # ---------------- small constants (DMA'd, not generated) -------
            cf = singles.tile([P, 2 * P], F32)
            nc.scalar.dma_start(cf[:], cid_f[:])
            id_bf = singles.tile([P, P], BF16)
            nc.scalar.dma_start(id_bf[:], cid_bf[:])

            enc_pre[0] = load_chunk(0)
            # Wenc load goes out right after chunk 0 so mm1 can start early
            we_f32 = []
            for oi in range(NH):
                t = wsetup.tile([P, H], F32, tag=f"wef32_{oi}")
                nc.scalar.dma_start(t[:], wenc[oi * P : (oi + 1) * P, :])
                we_f32.append(t)
            enc_pre[1] = load_chunk(1)
            dec_nat = singles.tile([1, H], F32)
            va_nat = singles.tile([1, H], F32)
            nc.scalar.dma_start(dec_nat[:], dec[:])
            nc.scalar.dma_start(va_nat[:], va[:])
            mask_nat = singles.tile([NST, P], F32)
            nc.scalar.dma_start(mask_nat[:], msk[:].rearrange("(j p) -> j p", p=P))

            id_f = cf[:, 0:P]
            one1 = cf[0:1, P : P + 1]
            ones_col = cf[:, P : P + 1]
            ones_row = cf[0:1, P : 2 * P]
            ones8 = cf[0:NCORES, P : P + 1]

            # ---------------- stage A: cast + transpose one chunk ----------
            def stage_a(c):
                enc_f32 = enc_pre.pop(c) if c in enc_pre else load_chunk(c)
                enc_bf = []
                for st in range(CH):
                    t = io.tile([P, H], BF16, tag="enc_bf")
                    nc.vector.tensor_copy(t[:], enc_f32[st][:])
                    enc_bf.append(t)
                encT = []
                for hi in range(NH):
                    ptt = pt_pool.tile([P, CH * P], BF16, tag="pt")
                    for st in range(CH):
                        nc.tensor.transpose(
                            ptt[:, st * P : (st + 1) * P],
                            enc_bf[st][:, hi * P : (hi + 1) * P],
                            id_bf[:],
                        )
                    t = io.tile([P, CH * P], BF16, tag="encT")
                    nc.vector.tensor_copy(t[:], ptt[:])
                    encT.append(t)
                return enc_bf, encT

            a_tiles = {0: stage_a(0), 1: stage_a(1)}

            # ---------------- remaining setup ----------------
            # Wenc -> WencT bf16 via PE transposes
            wencT = []
            for hi in range(NH):
                pw = pt_pool.tile([P, H], F32, tag="pt")
                for oi in range(NH):
                    nc.tensor.transpose(
                        pw[:, oi * P : (oi + 1) * P],
                        we_f32[oi][:, hi * P : (hi + 1) * P],
                        id_f[:],
                    )
                t = wsetup.tile([P, H], BF16, tag=f"weT_{hi}")
                nc.vector.tensor_copy(t[:], pw[:])
                wencT.append(t)

            # va columns [128, NH] bf16 via K=1 broadcast matmuls
            pc = pe_pool.tile([P, NH], F32, tag="pe")
            for hi in range(NH):
                nc.tensor.matmul(
                    pc[:, hi : hi + 1],
                    va_nat[0:1, hi * P : (hi + 1) * P],
                    one1[:],
                    start=True,
                    stop=True,
                )
            va_col = singles.tile([P, NH], BF16)
            nc.vector.tensor_copy(va_col[:], pc[:])

            # Wdec natural + temp_dec on DVE
            wd_f32 = []
            for oi in range(NH):
                t = wsetup.tile([P, H], F32, tag=f"wdf32_{oi}")
                nc.scalar.dma_start(t[:], wdec[oi * P : (oi + 1) * P, :])
                wd_f32.append(t)
            pdb = pe_pool.tile([P, H], F32, tag="pe")
            nc.tensor.matmul(pdb[:], ones_row[:], dec_nat[:], start=True, stop=True)
            dec_bcast = singles.tile([P, H], F32)
            nc.vector.tensor_copy(dec_bcast[:], pdb[:])
            td_col = singles.tile([P, NH], F32)
            tt_scratch = singles.tile([P, H], F32)
            for oi in range(NH):
                nc.vector.tensor_mul(tt_scratch[:], wd_f32[oi][:], dec_bcast[:])
                nc.vector.reduce_sum(
                    td_col[:, oi : oi + 1],
                    tt_scratch[:],
                    axis=mybir.AxisListType.X,
                )

            # mask: PE-transpose [64,128] -> [128,64]
            pm = pt_pool.tile([P, NST], F32, tag="pt")
            nc.tensor.transpose(pm[:], mask_nat[:], id_f[:NST, :NST])
            mask_sb = singles.tile([P, NST], F32)
            nc.vector.tensor_copy(mask_sb[:], pm[:])

            # persistent softmax state
            expm = singles.tile([P, NST], F32)  # masked exp(e)
            w_bf16 = singles.tile([P, NST], BF16)  # bf16 copy for ctx matmul
            ctx_psum = psum_ctx.tile([1, H], F32)
            ar_sb = singles.tile([1, AR_N], F32)
            nc.vector.memset(ar_sb[:], 0.0)

            # ---------------- stage B: matmuls + softmax pieces ------------
            def stage_b(c, enc_bf, encT):
                x_bf = []
                for oi in range(NH):
                    p1 = psum1.tile([P, CH * P], F32, tag="psum1")
                    for hi in range(NH):
                        nc.tensor.matmul(
                            p1[:],
                            wencT[hi][:, oi * P : (oi + 1) * P],
                            encT[hi][:],
                            start=(hi == 0),
                            stop=(hi == NH - 1),
                        )
                    xt = io.tile([P, CH * P], BF16, tag="x_bf")
                    nc.scalar.activation(
                        xt[:], p1[:], AF.Tanh, bias=td_col[:, oi : oi + 1]
                    )
                    x_bf.append(xt)
                # energies into [128, CH] columns (x blocks stationary)
                pe2 = pe_pool.tile([P, CH], F32, tag="pe")
                for st in range(CH):
                    for oi in range(NH):
                        nc.tensor.matmul(
                            pe2[:, st : st + 1],
                            x_bf[oi][:, st * P : (st + 1) * P],
                            va_col[:, oi : oi + 1],
                            start=(oi == 0),
                            stop=(oi == NH - 1),
                        )
                cols = slice(c * CH, (c + 1) * CH)
                nc.scalar.activation(expm[:, cols], pe2[:], AF.Exp)
                nc.vector.tensor_mul(expm[:, cols], expm[:, cols], mask_sb[:, cols])
                nc.vector.tensor_copy(w_bf16[:, cols], expm[:, cols])
                for st in range(CH):
                    j = c * CH + st
                    nc.tensor.matmul(
                        ctx_psum[:],
                        w_bf16[:, j : j + 1],
                        enc_bf[st][:],
                        start=(j == 0),
                        stop=(j == NST - 1),
                        skip_group_check=True,
                    )

            # ---------------- main loop, software-pipelined ---------------
            for c in range(NCHUNK):
                enc_bf, encT = a_tiles.pop(c)
                if c + 2 < NCHUNK:
                    a_tiles[c + 2] = stage_a(c + 2)
                stage_b(c, enc_bf, encT)

            # ---------------- tail ----------------
            zrow = singles.tile([P, 1], F32)
            nc.vector.reduce_sum(zrow[:], expm[:], axis=mybir.AxisListType.X)
            z_psum = pe_pool.tile([1, 1], F32, tag="pe")
            nc.tensor.matmul(z_psum[:], ones_col[:], zrow[:], start=True, stop=True)
            nc.vector.tensor_copy(ar_sb[0:1, 0:H], ctx_psum[:])
            nc.vector.tensor_copy(ar_sb[0:1, H : H + 1], z_psum[:])
            nc.sync.dma_start(ar_in[:], ar_sb[:])
            nc.gpsimd.collective_compute(
                COLLECTIVE,
                ALU.bypass if COLLECTIVE == "AllGather" else ALU.add,
                replica_groups=[list(range(NCORES))],
                ins=[ar_in[:]],
                outs=[ag_out[:]],
            )
            # transpose expm for the contiguous store; overlaps the collective
            pa = pt_pool.tile([NST, P], F32, tag="pt")
            nc.tensor.transpose(pa[:], expm[:], id_f[:])
            expm_T = singles.tile([NST, P], F32)
            nc.vector.tensor_copy(expm_T[:], pa[:])

            rz = singles.tile([1, 1], F32)
            ctx_sb = singles.tile([1, H], F32)
            if COLLECTIVE == "AllGather":
                g_sb = singles.tile([NCORES, AR_N], F32)
                nc.sync.dma_start(g_sb[:], ag_out[:])
                # reduce the 8 ranks' partials on PE
                gy = pe_pool.tile([1, H], F32, tag="pe")
                nc.tensor.matmul(
                    gy[:], ones8[:], g_sb[:, 0:H], start=True, stop=True
                )
                gz = pt_pool.tile([1, 1], F32, tag="pt")
                nc.tensor.matmul(
                    gz[:], ones8[:], g_sb[:, H : H + 1], start=True, stop=True
                )
                nc.vector.reciprocal(rz[:], gz[:])
                nc.vector.tensor_scalar_mul(ctx_sb[:], gy[:], rz[:])
            else:
                g_sb = singles.tile([1, AR_N], F32)
                nc.sync.dma_start(g_sb[:], ag_out[:])
                nc.vector.reciprocal(rz[:], g_sb[0:1, H : H + 1])
                nc.vector.tensor_scalar_mul(ctx_sb[:], g_sb[0:1, 0:H], rz[:])
            nc.sync.dma_start(ctx_out[:], ctx_sb[:])
            # w = expm/Z: broadcast 1/Z to 64 partitions, scale, store
            b_psum = pe_pool.tile([NST, 1], F32, tag="pe")
            nc.tensor.matmul(
                b_psum[:], ones_row[0:1, 0:NST], rz[:], start=True, stop=True
            )
            rz_col = singles.tile([NST, 1], F32)
            nc.vector.tensor_copy(rz_col[:], b_psum[:])
            w_T = singles.tile([NST, P], F32)
            nc.vector.tensor_scalar_mul(w_T[:], expm_T[:], rz_col[:])
            nc.sync.dma_start(attn_out[:].rearrange("(j p) -> j p", p=P), w_T[:])

    return _fix_wait_overflow(nc)


_NC = None


def _get_nc():
    global _NC
    if _NC is None:
        _NC = build_nc()
    return _NC


def make_in_maps(encoder_output, decoder_hidden, attention_mask, Wenc_w, Wdec_w, va_w):
    enc = np.ascontiguousarray(np.asarray(encoder_output, dtype=np.float32))[0]
    dec = np.ascontiguousarray(np.asarray(decoder_hidden, dtype=np.float32))
    mask_f = np.asarray(attention_mask).astype(np.float32)
    wenc = np.ascontiguousarray(np.asarray(Wenc_w, dtype=np.float32))
    wdec = np.ascontiguousarray(np.asarray(Wdec_w, dtype=np.float32))
    va = np.ascontiguousarray(np.asarray(va_w, dtype=np.float32))

    cid_f = np.zeros((P, 2 * P), dtype=np.float32)
    cid_f[:, :P] = np.eye(P, dtype=np.float32)
    cid_f[:, P:] = 1.0
    cid_bf = np.eye(P, dtype=ml_dtypes.bfloat16)

    in_maps = []
    for r in range(NCORES):
        sl = slice(r * SLOC, (r + 1) * SLOC)
        in_maps.append(
            {
                "enc": np.ascontiguousarray(enc[sl]),
                "dec": dec,
                "msk": np.ascontiguousarray(mask_f[sl]),
                "wenc": wenc,
                "wdec": wdec,
                "va": va,
                "cid_f": cid_f,
                "cid_bf": cid_bf,
            }
        )
    return in_maps


def kernel(encoder_output, decoder_hidden, attention_mask, Wenc_w, Wdec_w, va_w):
    in_maps = make_in_maps(
        encoder_output, decoder_hidden, attention_mask, Wenc_w, Wdec_w, va_w
    )
    res = run_bass_kernel_spmd(_get_nc(), in_maps, list(range(NCORES)))

    attn = np.concatenate(
        [np.asarray(res.results[r]["attn_out"]).reshape(-1) for r in range(NCORES)]
    ).reshape(1, S, 1)
    ctx = np.asarray(res.results[0]["ctx_out"]).reshape(1, 1, H)
    return ctx, attn
